# revision 11
# baseline (speedup 1.0000x reference)
"""Masked mean-pool (NonZeroAvgPool) Trainium2 Bass kernel.

out[b, d] = sum_s (tokens[b,s] != 0) * x[b,s,d] / sum_s (tokens[b,s] != 0)

Full shapes: x [16, 4096, 512] f32, tokens [16, 4096] i32 -> out [16, 512] f32.
Sharding: pure data parallel over batch; 2 batches per core on 8 cores.

Per-core program (shapes [2, 4096, 512] / [2, 4096] -> [2, 512]):
  - sequence rows are indexed s = p*32 + c  (p: SBUF partition, c: chunk)
    so every DMA is contiguous per partition.
  - valid[p, c] = (tokens != 0) as f32 via DVE not_equal
  - count      = ones[128,1].T @ rowsum(valid)        (PE, [1,1] PSUM)
  - num[1, D]  = sum_c valid[:, c].T @ x_tile[:, c, :] (PE, accumulated in PSUM)
  - out row    = num * (1/count)                       (DVE), then a 2KB store
    per batch (b0's store overlaps b1's stream).

x streams into two static full-batch SBUF tiles (no ring, no WAR hazards).
Batch 0 is one 8MB DMA; batch 1 tapers (17,8,4,2,1).  Engines drain the
ring FIFO back-to-back, so b0's single DMA completes ~60% into the stream
and its whole 32-matmul burst (including the cold-HAM warmup) hides under
b1's transfers; b1's matmuls then track its taper closely, and after the
last byte lands the critical path is one matmul + divide + 2KB store.
Timeline (ns, good run): 0-6k runtime preamble, 6-9k BB prologue + first
descriptors, stream 9k-49k, tail ~49-51k, epilogue ~2.5k.

Hard-won constraints baked in here:
  - DMA partition dim must be 128 (or at least 16-divisible): the
    descriptor spray across the 16 SDMA engines splits the outer AP dim by
    its largest power-of-2 factor; a 124-partition DMA collapsed onto 4
    engines (measured 3x slowdown).
  - Per-descriptor engine overhead ~15ns: fewer+bigger descriptors win
    (one 64KB descriptor per partition for b0).
  - >8 total DMAs reuses DMAHW sem lanes: usually works, but crashed
    ~1-in-5 runs with NRT_EXEC_UNIT_UNRECOVERABLE.  Stay at 8.
  - SDMA engine 15 intermittently runs ~19% slow (seen in most runs,
    cause external - possibly profiler/neighbor traffic); the spray gives
    every engine 1/16 of every DMA, so it stretches the stream and there
    is no layout that starves it while keeping the 16-way spray.
"""

import os
from contextlib import ExitStack

import numpy as np

import concourse.bacc as bacc
import concourse.bass as bass
import concourse.tile as tile
from concourse import mybir
from concourse.bass_utils import run_bass_kernel_spmd

B, S, D = 16, 4096, 512
NCORES = 8
BPC = B // NCORES  # batches per core = 2
P = 128            # SBUF partitions
CPB = S // P       # chunks per batch = 32

# Chunks-per-dma_start, per batch.  ASYMMETRIC taper: batch 0 loads as one
# 8MB DMA (fewest descriptors; its 32-matmul burst runs entirely under
# batch 1's stream), while batch 1 tapers 17,8,4,2,1 so the PE tracks the
# stream closely and the last completion sem gates a single matmul.
# Total DMA count stays at 8 (1 tok + 6 x + 1 out): >8 DMAs reuse the 8
# DMAHW completion-sem lanes, which works MOST of the time but crashed
# ~1-in-5 runs with NRT_EXEC_UNIT_UNRECOVERABLE in testing - not worth it.
GROUPS = [
    [int(g) for g in part.split(",")]
    for part in os.environ.get("K_GROUPS", "32/17,8,4,2,1").split("/")
]
assert all(sum(gs) == CPB for gs in GROUPS) and len(GROUPS) == BPC
X_ENGINE = os.environ.get("K_XENG", "act")  # sync | act | gpsimd

_NC = None


def _build_nc():
    # Bacc (not plain Bass): its compile() runs generate_event_semaphores,
    # which splits multi-wait instructions onto InstEventSemaphore — TRN2
    # instructions can carry at most one sem wait.
    nc = bacc.Bacc(trn_type="TRN2")
    x = nc.dram_tensor("x", [BPC, S, D], mybir.dt.float32, kind="ExternalInput")
    tokens = nc.dram_tensor("tokens", [BPC, S], mybir.dt.int32, kind="ExternalInput")
    out = nc.dram_tensor("out", [BPC, D], mybir.dt.float32, kind="ExternalOutput")

    # s = p*CPB + c : per-partition contiguous rows
    xa = x[:].rearrange("b (p c) d -> b p c d", p=P)   # [BPC, 128, 32, 512]
    ta = tokens[:].rearrange("b (p c) -> p b c", p=P)  # [128, BPC, 32]
    oa = out[:].rearrange("b d -> (b d)")              # [BPC*512]

    with TileKernel(nc) as tk:
        tk.body(xa, ta, oa)
    nc.compile()
    return nc


class TileKernel:
    def __init__(self, nc):
        self.nc = nc
        self.ctx = ExitStack()
        self.tc = None

    def __enter__(self):
        self.tc = self.ctx.enter_context(tile.TileContext(self.nc))
        return self

    def __exit__(self, *exc):
        return self.ctx.__exit__(*exc)

    def body(self, xa, ta, oa):
        nc = self.nc
        tc = self.tc
        ctx = self.ctx

        xpool = ctx.enter_context(tc.tile_pool(name="xpool", bufs=1))
        vpool = ctx.enter_context(tc.tile_pool(name="vpool", bufs=1))
        spool = ctx.enter_context(tc.tile_pool(name="spool", bufs=2))
        singles = ctx.enter_context(tc.tile_pool(name="singles", bufs=1))
        psum = ctx.enter_context(tc.tile_pool(name="psum", bufs=2, space="PSUM"))

        xeng = {"sync": nc.sync, "act": nc.scalar, "gpsimd": nc.gpsimd}[X_ENGINE]

        # --- x streams first: every DMA writes its own region of a static
        # full-batch tile exactly once.  float32r: the DMA is a pure bit
        # copy; single-pass fp32 matmul (4x faster than fp32's two half-rate
        # passes); mask weights are exact 0/1, PSUM accumulates in fp32.
        xb = [
            xpool.tile([P, CPB, D], mybir.dt.float32r, name=f"xb{b}")
            for b in range(BPC)
        ]
        for b in range(BPC):
            c0 = 0
            for g in GROUPS[b]:
                xeng.dma_start(
                    out=xb[b][:, c0:c0 + g, :],
                    in_=xa[b, :, c0:c0 + g, :].bitcast(mybir.dt.float32r),
                )
                c0 += g

        # --- mask + counts for both batches (one tok DMA) --------------------
        tok = vpool.tile([P, BPC, CPB], mybir.dt.int32)
        nc.sync.dma_start(out=tok, in_=ta)
        # valid is declared float32r so the fp32r matmul's verifier sees a
        # rounded producer; its values (0.0/1.0) are exact in any precision.
        valid = vpool.tile([P, BPC, CPB], mybir.dt.float32r)
        nc.vector.tensor_scalar(
            out=valid, in0=tok, scalar1=0, scalar2=None,
            op0=mybir.AluOpType.not_equal,
        )
        rowsum = spool.tile([P, BPC], mybir.dt.float32)
        nc.vector.reduce_sum(
            out=rowsum, in_=valid.bitcast(mybir.dt.float32),
            axis=mybir.AxisListType.X,
        )

        ones = singles.tile([P, 1], mybir.dt.float32)
        nc.vector.memset(ones, 1.0)

        orow = [
            spool.tile([1, D], mybir.dt.float32, name=f"orow{b}")
            for b in range(BPC)
        ]

        for b in range(BPC):
            cnt = psum.tile([1, 1], mybir.dt.float32)
            nc.tensor.matmul(cnt, ones, rowsum[:, b:b + 1], start=True, stop=True)
            recip = spool.tile([1, 1], mybir.dt.float32)
            nc.vector.reciprocal(recip, cnt)

            # --- masked sum: one matmul per chunk, gated by its group's DMA.
            num = psum.tile([1, D], mybir.dt.float32)
            for c in range(CPB):
                nc.tensor.matmul(
                    num, valid[:, b, c:c + 1], xb[b][:, c, :],
                    start=(c == 0), stop=(c == CPB - 1),
                )

            # --- divide + store: b0's store overlaps b1's stream; only b1's
            # 2KB store sits on the tail.  The divide is split across DVE and
            # ACT so the two halves run in parallel (~0.45us instead of 0.74).
            h = D // 2
            nc.vector.tensor_scalar_mul(orow[b][:, :h], num[:, :h], recip)
            nc.scalar.mul(orow[b][:, h:], num[:, h:], recip)
            nc.sync.dma_start(out=oa[b * D:(b + 1) * D], in_=orow[b])


def _get_nc():
    global _NC
    if _NC is None:
        _NC = _build_nc()
    return _NC


def _shard(x, tokens):
    x = np.ascontiguousarray(np.asarray(x, dtype=np.float32))
    tokens = np.ascontiguousarray(np.asarray(tokens, dtype=np.int32))
    return [
        {
            "x": x[c * BPC:(c + 1) * BPC],
            "tokens": tokens[c * BPC:(c + 1) * BPC],
        }
        for c in range(NCORES)
    ]


def kernel(x, tokens):
    res = run_bass_kernel_spmd(_get_nc(), _shard(x, tokens), core_ids=list(range(NCORES)))
    return np.concatenate([r["out"] for r in res.results], axis=0)


def _install_ntff_shim():
    """The agent image's antenv lacks axon_hooks, so bass_utils' trace path
    can't find the NTFF hook. Recreate the tiny get/set module and register
    trn_boot's ctypes-based hook against the injected libaxon_pjrt.so."""
    import sys
    import types

    if "antenv.axon_hooks" in sys.modules:
        return
    mod = types.ModuleType("antenv.axon_hooks")
    state = {"hook": None}
    mod.set_axon_ntff_profile_hook = lambda h: state.__setitem__("hook", h)
    mod.get_axon_ntff_profile_hook = lambda: state["hook"]
    sys.modules["antenv.axon_hooks"] = mod
    try:
        from trn_agent_boot.trn_boot import _ntff_profile_via_ctypes

        mod.set_axon_ntff_profile_hook(
            _ntff_profile_via_ctypes("/opt/axon/libaxon_pjrt.so")
        )
    except Exception:
        pass


def kernel_profiled(x, tokens):
    """Same as kernel() but with NTFF tracing; returns (out, BassKernelResults)."""
    _install_ntff_shim()
    res = run_bass_kernel_spmd(
        _get_nc(), _shard(x, tokens), core_ids=list(range(NCORES)), trace=True
    )
    out = np.concatenate([r["out"] for r in res.results], axis=0)
    return out, res
